# revision 9
# baseline (speedup 1.0000x reference)
# Grouped-GEMM MoE (8 experts, top-2, SwiGLU) on 8 Trainium2 NeuronCores.
#
# Strategy (expert-parallel, host-side all-to-all):
#   - Host routes tokens to experts. Tokens whose top-2 picks the same
#     expert twice are DEDUPLICATED (y is identical for both slots; the
#     combine weight becomes ew1+ew2) — ~1/8 of tokens, so the per-expert
#     batch shrinks accordingly.
#   - Core e runs expert e's dense MLP feature-major: every matmul contracts
#     over the partition dim, the token dim always rides the free dim, so no
#     on-device transposes are needed:
#         H.T  = W1.T-tiles @ X.T      (bf16, K=2048)
#         actT = silu(Ha + b1a) * (Hb + b1b)   (ACT + DVE, fused bias)
#         Y.T  = W2.T-tiles @ actT     (bf16, K=2816)
#   - Host scatters Y rows back to tokens and does the weighted top-k
#     combine.
#
# Weights stream from HBM exactly once per call (pre-tiled host-side so each
# SBUF strip is one contiguous read); activations stay resident.

import os

import ml_dtypes
import numpy as np

import concourse.bacc as bacc
import concourse.mybir as mybir
import concourse.tile as tile
from concourse import bass_utils
from concourse.bass import ts

P = 128
C_MAX = 1536      # max token capacity per wave (SBUF-resident xt + act)

f32 = mybir.dt.float32
bf16 = mybir.dt.bfloat16
Silu = mybir.ActivationFunctionType.Silu
Identity = mybir.ActivationFunctionType.Identity
Alu = mybir.AluOpType

_cache = {}

# set by the most recent kernel() call when KERNEL_TRACE=1 (test harness use)
last_exec_time_ns = None
last_results = None


def _chunking(cmax):
    """Pick (chunk_width, n_chunks): CW <= 512 (PSUM bank), minimal padded
    capacity, CW large enough that weight loads stay hidden."""
    NCH = max(1, -(-cmax // 512))
    cw = max(256, ((-(-cmax // NCH)) + 3) // 4 * 4)
    return cw, NCH


def _build(CW, NCH, H, F, n_cores):
    """Build+schedule the per-core MLP program for token capacity CW*NCH."""
    C = CW * NCH
    F2 = 2 * F
    KT1 = H // P      # k-tiles of GEMM1 (16)
    FT = F2 // P      # f-tiles of W1    (44)
    FP = F // P       # f-tile pairs / k-tiles of GEMM2 (22)
    MT = H // P       # m-tiles of GEMM2 (16)

    nc = bacc.Bacc("TRN2", target_bir_lowering=False, debug=False,
                   num_devices=n_cores)

    xt_d = nc.dram_tensor("xt", (H, C), bf16, kind="ExternalInput").ap()
    # pre-tiled weights: w1t[ft, p, ko, fi] = w1[ko*P + p, ft*P + fi]
    w1_d = nc.dram_tensor("w1t", (FT, P, KT1, P), bf16, kind="ExternalInput").ap()
    b1_d = nc.dram_tensor("b1", (F2, 1), f32, kind="ExternalInput").ap()
    # w2t[mt, p, ko, mi] = w2[ko*P + p, mt*P + mi]  (bf16)
    w2_d = nc.dram_tensor("w2t", (MT, P, FP, P), bf16, kind="ExternalInput").ap()
    b2_d = nc.dram_tensor("b2", (H, 1), f32, kind="ExternalInput").ap()
    yt_d = nc.dram_tensor("yt", (H, C), f32, kind="ExternalOutput").ap()

    xt_t = xt_d.rearrange("(ko p) c -> p ko c", p=P)    # [128, KT1, C]
    b1_t = b1_d.rearrange("(ft p) one -> p ft one", p=P)
    b2_t = b2_d.rearrange("(mt p) one -> p mt one", p=P)
    yt_t = yt_d.rearrange("(mt p) c -> p mt c", p=P)

    # Stagger chunk ch by LAG f-iterations behind chunk 0 so the startup
    # only waits on chunk 0's xt tiles + the first weight strips (the DMA
    # queues deliver xt chunk 1+ and later weights while PE is busy).
    LAG = 4
    order = []
    for step in range(FP + (NCH - 1) * LAG):
        for ch in range(NCH):
            f = step - ch * LAG
            if 0 <= f < FP:
                order.append((f, ch))

    with tile.TileContext(nc) as tc:
        with tc.tile_pool(name="persist", bufs=1) as persist, \
             tc.tile_pool(name="w1pool", bufs=2 * (LAG + 3)) as w1pool, \
             tc.tile_pool(name="w2pool", bufs=2) as w2pool, \
             tc.tile_pool(name="evac", bufs=3) as evac, \
             tc.tile_pool(name="ps1", bufs=2, space="PSUM") as ps1, \
             tc.tile_pool(name="ps2", bufs=4, space="PSUM") as ps2:

            # chunk 0's xt tiles stream on their own queue at t=0; later
            # chunks are only needed LAG f-iterations in, so they queue up
            # behind the first w1 strips on the weight queue (see below).
            xt_sb = {}
            for k in range(KT1):
                t = persist.tile([P, CW], bf16, tag=f"xt_{k}_0")
                nc.gpsimd.dma_start(t[:], xt_t[:, k, ts(0, CW)])
                xt_sb[k, 0] = t
            for ch in range(1, NCH):
                for k in range(KT1):
                    t = persist.tile([P, CW], bf16, tag=f"xt_{k}_{ch}")
                    xt_sb[k, ch] = t
            act_sb = persist.tile([P, FP, C], bf16)
            b1_sb = persist.tile([P, FT, 1], f32)
            nc.sync.dma_start(b1_sb[:], b1_t)
            b2_sb = persist.tile([P, MT, 1], f32)
            nc.sync.dma_start(b2_sb[:], b2_t)

            # ---- GEMM1 + SwiGLU, one f-tile pair (a, b halves) at a time
            with nc.named_scope("gemm1"):
                w1_sb = {}
                n_pairs = 0
                for fp, ch in order:
                    if fp not in w1_sb:
                        w1a = w1pool.tile([P, KT1, P], bf16, tag="w1s")
                        w1b = w1pool.tile([P, KT1, P], bf16, tag="w1s")
                        nc.scalar.dma_start(w1a[:], w1_d[fp])
                        nc.scalar.dma_start(w1b[:], w1_d[FP + fp])
                        w1_sb[fp] = (w1a, w1b)
                        n_pairs += 1
                        if n_pairs == 3:
                            # xt chunks 1+ ride the weight queue behind the
                            # first two pairs (needed from step LAG onward)
                            for ch2 in range(1, NCH):
                                for k in range(KT1):
                                    nc.scalar.dma_start(
                                        xt_sb[k, ch2][:], xt_t[:, k, ts(ch2, CW)])
                    w1a, w1b = w1_sb[fp]
                    pa = ps1.tile([P, CW], f32, tag="pa")
                    pb = ps1.tile([P, CW], f32, tag="pb")
                    for k in range(KT1):
                        nc.tensor.matmul(pa, w1a[:, k],
                                         xt_sb[k, ch][:],
                                         start=(k == 0), stop=(k == KT1 - 1))
                    for k in range(KT1):
                        nc.tensor.matmul(pb, w1b[:, k],
                                         xt_sb[k, ch][:],
                                         start=(k == 0), stop=(k == KT1 - 1))
                    s = evac.tile([P, CW], f32, tag="silu")
                    nc.scalar.activation(s, pa, Silu, bias=b1_sb[:, fp])
                    # act = (pb + b1b) * silu(pa + b1a), cast bf16 on write
                    nc.vector.scalar_tensor_tensor(
                        act_sb[:, fp, ts(ch, CW)], pb, b1_sb[:, FP + fp], s,
                        Alu.add, Alu.mult)
                    if ch == NCH - 1:
                        del w1_sb[fp]

            # ---- GEMM2
            with nc.named_scope("gemm2"):
                for m in range(MT):
                    # same queue as w1 so the w2 prefetch queues up BEHIND the
                    # startup-critical w1 strips instead of competing at t=0
                    w2s = w2pool.tile([P, FP, P], bf16, tag="w2s")
                    nc.scalar.dma_start(w2s[:], w2_d[m])
                    for ch in range(NCH):
                        py = ps2.tile([P, CW], f32, tag="py")
                        for k in range(FP):
                            nc.tensor.matmul(py, w2s[:, k],
                                             act_sb[:, k, ts(ch, CW)],
                                             start=(k == 0), stop=(k == FP - 1))
                        y = evac.tile([P, CW], f32, tag="y")
                        if m == MT - 1 and ch == NCH - 1:
                            # split the very last evac so ACT/DVE + the two DMA
                            # queues pipeline the kernel tail
                            h = CW // 2
                            nc.scalar.activation(y[:, :h], py[:, :h], Identity,
                                                 bias=b2_sb[:, m])
                            nc.vector.tensor_scalar_add(y[:, h:], py[:, h:],
                                                        b2_sb[:, m])
                            nc.sync.dma_start(
                                yt_t[:, m, ts(ch, CW)][:, :h], y[:, :h])
                            nc.gpsimd.dma_start(
                                yt_t[:, m, ts(ch, CW)][:, h:], y[:, h:])
                        else:
                            nc.scalar.activation(y, py, Identity,
                                                 bias=b2_sb[:, m])
                            nc.sync.dma_start(yt_t[:, m, ts(ch, CW)], y[:])

    nc.compile()
    return nc


def kernel(hidden_states, expert_weights, w1, b1, w2, b2, top_experts):
    global last_exec_time_ns, last_results

    hidden_states = np.asarray(hidden_states)
    B, S, H = hidden_states.shape
    E, _, F2 = np.asarray(w1).shape
    F = F2 // 2
    topk = np.asarray(top_experts).shape[-1]
    N = B * S
    n_cores = 8
    assert E == n_cores, f"kernel assumes one expert per core, got E={E}"

    x = np.ascontiguousarray(hidden_states.reshape(N, H).astype(np.float32))
    te = np.asarray(top_experts).reshape(N, topk).astype(np.int64)
    ew = np.asarray(expert_weights).reshape(N, topk).astype(np.float32)

    # Dedup routing: token t needs expert e once, with combined weight
    # sum_j ew[t, j] * [te[t, j] == e].
    tok_lists = []
    wt_lists = []
    for e in range(E):
        sel = (te == e)                    # [N, topk]
        toks = np.nonzero(sel.any(axis=1))[0]
        wts = (ew[toks] * sel[toks]).sum(axis=1)
        tok_lists.append(toks)
        wt_lists.append(wts)
    counts = np.array([len(t) for t in tok_lists])

    cmax = int(counts.max())
    cw, nch = _chunking(min(max(cmax, 1), C_MAX))
    cap = cw * nch
    n_waves = max(1, -(-cmax // cap))

    key = (int(cw), int(nch), H, F, n_cores)
    if key not in _cache:
        _cache[key] = _build(*key)
    nc = _cache[key]

    # per-expert constant inputs (weights pre-tiled to contiguous SBUF strips)
    KT1, FT, FP, MT = H // P, F2 // P, F // P, H // P
    const_maps = []
    for e in range(E):
        w1e = np.asarray(w1[e], dtype=np.float32).astype(ml_dtypes.bfloat16)
        w1t = np.ascontiguousarray(
            w1e.reshape(KT1, P, FT, P).transpose(2, 1, 0, 3))
        w2e = np.asarray(w2[e], dtype=np.float32).astype(ml_dtypes.bfloat16)
        w2t = np.ascontiguousarray(
            w2e.reshape(FP, P, MT, P).transpose(2, 1, 0, 3))
        const_maps.append({
            "w1t": w1t,
            "b1": np.ascontiguousarray(
                np.asarray(b1[e], dtype=np.float32).reshape(F2, 1)),
            "w2t": w2t,
            "b2": np.ascontiguousarray(
                np.asarray(b2[e], dtype=np.float32).reshape(H, 1)),
        })

    trace = os.environ.get("KERNEL_TRACE", "") == "1"
    out = np.zeros((N, H), dtype=np.float32)
    last_results = []
    for w in range(n_waves):
        in_maps = []
        for e in range(E):
            lo = w * cap
            toks = tok_lists[e][lo: lo + cap]
            xt = np.zeros((H, cap), dtype=ml_dtypes.bfloat16)
            if len(toks):
                xt[:, :len(toks)] = x[toks].T.astype(ml_dtypes.bfloat16)
            in_maps.append({"xt": xt, **const_maps[e]})
        tmpdir = None
        if trace:
            import shutil
            tmpdir = f"/tmp/moe_trace_w{w}"
            shutil.rmtree(tmpdir, ignore_errors=True)
            os.makedirs(tmpdir, exist_ok=True)
        res = bass_utils.run_bass_kernel_spmd(
            nc, in_maps, core_ids=list(range(n_cores)), trace=trace,
            tmpdir=tmpdir)
        last_results.append(res)
        if trace:
            last_exec_time_ns = res.exec_time_ns
        for e in range(E):
            lo = w * cap
            toks = tok_lists[e][lo: lo + cap]
            if len(toks):
                yt = res.results[e]["yt"]
                wts = wt_lists[e][lo: lo + cap]
                out[toks] += wts[:, None] * yt[:, :len(toks)].T

    return out.reshape(B, S, H).astype(np.float32)


# revision 27
# speedup vs baseline: 1.0144x; 1.0144x over previous
# Grouped-GEMM MoE (8 experts, top-2, SwiGLU) on 8 Trainium2 NeuronCores.
#
# Strategy (expert-parallel, host-side all-to-all):
#   - Host routes tokens to experts. Tokens whose top-2 picks the same
#     expert twice are DEDUPLICATED (y is identical for both slots; the
#     combine weight becomes ew1+ew2) — ~1/8 of tokens, so the per-expert
#     batch shrinks accordingly.
#   - Core e runs expert e's dense MLP feature-major: every matmul contracts
#     over the partition dim, the token dim always rides the free dim, so no
#     on-device transposes are needed:
#         H.T  = W1.T-tiles @ X.T      (bf16, K=2048)
#         actT = silu(Ha + b1a) * (Hb + b1b)   (ACT + DVE, fused bias)
#         Y.T  = W2.T-tiles @ actT     (bf16, K=2816)
#   - Host scatters Y rows back to tokens and does the weighted top-k
#     combine.
#
# Weights stream from HBM exactly once per call (pre-tiled host-side so each
# SBUF strip is one contiguous read); activations stay resident.

import os

import ml_dtypes
import numpy as np

import concourse.bacc as bacc
import concourse.mybir as mybir
import concourse.tile as tile
from concourse import bass_utils
from concourse.bass import ts

P = 128
C_MAX = 1536      # max token capacity per wave (SBUF-resident xt + act)

f32 = mybir.dt.float32
bf16 = mybir.dt.bfloat16
Silu = mybir.ActivationFunctionType.Silu
Identity = mybir.ActivationFunctionType.Identity
Alu = mybir.AluOpType

_cache = {}

# set by the most recent kernel() call when KERNEL_TRACE=1 (test harness use)
last_exec_time_ns = None
last_results = None


def _chunking(cmax):
    """Pick (chunk_width, n_chunks): CW <= 512 (PSUM bank), minimal padded
    capacity, CW large enough that weight loads stay hidden."""
    NCH = max(1, -(-cmax // 512))
    cw = max(256, ((-(-cmax // NCH)) + 3) // 4 * 4)
    return cw, NCH


def _build(CW, NCH, H, F, n_cores):
    """Build+schedule the per-core MLP program for token capacity CW*NCH."""
    C = CW * NCH
    F2 = 2 * F
    KT1 = H // P      # k-tiles of GEMM1 (16)
    FT = F2 // P      # f-tiles of W1    (44)
    FP = F // P       # f-tile pairs / k-tiles of GEMM2 (22)
    MT = H // P       # m-tiles of GEMM2 (16)

    nc = bacc.Bacc("TRN2", target_bir_lowering=False, debug=False,
                   num_devices=n_cores)

    xt_d = nc.dram_tensor("xt", (H, C), bf16, kind="ExternalInput").ap()
    # pre-tiled weights: w1t[ft, p, ko, fi] = w1[ko*P + p, ft*P + fi]
    w1_d = nc.dram_tensor("w1t", (FT, P, KT1, P), bf16, kind="ExternalInput").ap()
    # biases pre-tiled host-side to [P, n] so the DMA descriptors are
    # per-partition contiguous: b1t[p, ft] = b1[ft*P + p]
    b1_d = nc.dram_tensor("b1t", (P, FT), f32, kind="ExternalInput").ap()
    # w2t[mt, p, ko, mi] = w2[ko*P + p, mt*P + mi]  (bf16)
    w2_d = nc.dram_tensor("w2t", (MT, P, FP, P), bf16, kind="ExternalInput").ap()
    b2_d = nc.dram_tensor("b2t", (P, MT), f32, kind="ExternalInput").ap()
    yt_d = nc.dram_tensor("yt", (H, C), f32, kind="ExternalOutput").ap()

    xt_t = xt_d.rearrange("(ko p) c -> p ko c", p=P)    # [128, KT1, C]
    yt_t = yt_d.rearrange("(mt p) c -> p mt c", p=P)

    # Stagger chunk ch by LAG f-iterations behind chunk 0 so the startup
    # only waits on chunk 0's xt tiles + the first weight strips (the DMA
    # queues deliver xt chunk 1+ and later weights while PE is busy).
    # Smaller LAG at NCH>=3 keeps the w1 pair-pinning window (and so the
    # weight pool) small enough for SBUF.
    LAG = 4 if NCH <= 2 else 2
    order = []
    for step in range(FP + (NCH - 1) * LAG):
        for ch in range(NCH):
            f = step - ch * LAG
            if 0 <= f < FP:
                order.append((f, ch))

    with tile.TileContext(nc) as tc:
        with tc.tile_pool(name="persist", bufs=1) as persist, \
             tc.tile_pool(name="w1pool", bufs=2 * ((NCH - 1) * LAG + 3)) as w1pool, \
             tc.tile_pool(name="w2pool", bufs=2) as w2pool, \
             tc.tile_pool(name="evac", bufs=3) as evac, \
             tc.tile_pool(name="ps1", bufs=2, space="PSUM") as ps1, \
             tc.tile_pool(name="ps2", bufs=4, space="PSUM") as ps2:

            # xt tiles in groups of GK k-tiles per DMA (fewer serialized
            # trigger slots). Chunk 0 streams on the two otherwise-idle HWDGE
            # queues (sync + vector) at t=0; later chunks are only needed LAG
            # f-iterations in, so they queue up behind the first w1 strips on
            # the weight queue (see below).
            GK = 4
            NG = KT1 // GK
            xt_sb = {}
            xt_groups = {}
            for g in range(NG):
                t = persist.tile([P, GK, CW], bf16, tag=f"xtg_{g}_0")
                nc.sync.dma_start(t[:], xt_t[:, ts(g, GK), ts(0, CW)])
                xt_groups[g, 0] = t
                for j in range(GK):
                    xt_sb[g * GK + j, 0] = t[:, j]
            for ch in range(1, NCH):
                for g in range(NG):
                    t = persist.tile([P, GK, CW], bf16, tag=f"xtg_{g}_{ch}")
                    xt_groups[g, ch] = t
                    for j in range(GK):
                        xt_sb[g * GK + j, ch] = t[:, j]
            act_sb = persist.tile([P, FP, C], bf16)
            b1_sb = persist.tile([P, FT], f32)
            b2_sb = persist.tile([P, MT], f32)

            # ---- GEMM1 + SwiGLU, one f-tile pair (a, b halves) at a time
            with nc.named_scope("gemm1"):
                w1_sb = {}
                n_pairs = 0
                for fp, ch in order:
                    if fp not in w1_sb:
                        w1a = w1pool.tile([P, KT1, P], bf16, tag="w1s")
                        w1b = w1pool.tile([P, KT1, P], bf16, tag="w1s")
                        if n_pairs < 1:
                            # halve the first pair's strips so the first
                            # matmul chain starts as soon as k-tiles land;
                            # a/b interleaved to match the chain order below
                            hk = KT1 // 2
                            for q in range(2):
                                nc.scalar.dma_start(w1a[:, ts(q, hk)],
                                                    w1_d[fp][:, ts(q, hk)])
                                nc.scalar.dma_start(w1b[:, ts(q, hk)],
                                                    w1_d[FP + fp][:, ts(q, hk)])
                        else:
                            nc.scalar.dma_start(w1a[:], w1_d[fp])
                            nc.scalar.dma_start(w1b[:], w1_d[FP + fp])
                        w1_sb[fp] = (w1a, w1b)
                        n_pairs += 1
                        if n_pairs == 1:
                            # biases ride behind the first pair (first needed
                            # by the fp=0 SwiGLU, ~15us in)
                            nc.sync.dma_start(b1_sb[:], b1_d)
                            nc.sync.dma_start(b2_sb[:], b2_d)
                        if n_pairs == 2:
                            # xt chunks 1+ ride the weight queue behind the
                            # first two pairs (needed from step LAG onward)
                            for ch2 in range(1, NCH):
                                for g in range(NG):
                                    nc.scalar.dma_start(
                                        xt_groups[g, ch2][:],
                                        xt_t[:, ts(g, GK), ts(ch2, CW)])
                    w1a, w1b = w1_sb[fp]
                    pa = ps1.tile([P, CW], f32, tag="pa")
                    pb = ps1.tile([P, CW], f32, tag="pb")
                    if fp == 0 and ch == 0:
                        # startup: interleave the a/b chains per xt group so
                        # each arriving group feeds 2*GK matmuls and PE never
                        # outruns the DMA pipe
                        for g in range(NG):
                            for half, p_ in ((w1a, pa), (w1b, pb)):
                                for k in range(g * GK, (g + 1) * GK):
                                    nc.tensor.matmul(
                                        p_, half[:, k], xt_sb[k, ch][:],
                                        start=(k == 0), stop=(k == KT1 - 1))
                    else:
                        for k in range(KT1):
                            nc.tensor.matmul(pa, w1a[:, k],
                                             xt_sb[k, ch][:],
                                             start=(k == 0), stop=(k == KT1 - 1))
                        for k in range(KT1):
                            nc.tensor.matmul(pb, w1b[:, k],
                                             xt_sb[k, ch][:],
                                             start=(k == 0), stop=(k == KT1 - 1))
                    s = evac.tile([P, CW], f32, tag="silu")
                    nc.scalar.activation(s, pa, Silu,
                                         bias=b1_sb[:, fp:fp + 1])
                    # act = (pb + b1b) * silu(pa + b1a), cast bf16 on write
                    nc.vector.scalar_tensor_tensor(
                        act_sb[:, fp, ts(ch, CW)], pb,
                        b1_sb[:, FP + fp:FP + fp + 1], s,
                        Alu.add, Alu.mult)
                    if ch == NCH - 1:
                        del w1_sb[fp]

            # ---- GEMM2
            with nc.named_scope("gemm2"):
                for m in range(MT):
                    # same queue as w1 so the w2 prefetch queues up BEHIND the
                    # startup-critical w1 strips instead of competing at t=0
                    w2s = w2pool.tile([P, FP, P], bf16, tag="w2s")
                    nc.scalar.dma_start(w2s[:], w2_d[m])
                    for ch in range(NCH):
                        py = ps2.tile([P, CW], f32, tag="py")
                        for k in range(FP):
                            nc.tensor.matmul(py, w2s[:, k],
                                             act_sb[:, k, ts(ch, CW)],
                                             start=(k == 0), stop=(k == FP - 1))
                        y = evac.tile([P, CW], f32, tag="y")
                        nc.scalar.activation(y, py, Identity,
                                             bias=b2_sb[:, m:m + 1])
                        nc.sync.dma_start(yt_t[:, m, ts(ch, CW)], y[:])

    nc.compile()
    return nc


def kernel(hidden_states, expert_weights, w1, b1, w2, b2, top_experts):
    global last_exec_time_ns, last_results

    hidden_states = np.asarray(hidden_states)
    B, S, H = hidden_states.shape
    E, _, F2 = np.asarray(w1).shape
    F = F2 // 2
    topk = np.asarray(top_experts).shape[-1]
    N = B * S
    n_cores = 8
    assert E == n_cores, f"kernel assumes one expert per core, got E={E}"

    x = np.ascontiguousarray(hidden_states.reshape(N, H).astype(np.float32))
    te = np.asarray(top_experts).reshape(N, topk).astype(np.int64)
    ew = np.asarray(expert_weights).reshape(N, topk).astype(np.float32)

    # Dedup routing: token t needs expert e once, with combined weight
    # sum_j ew[t, j] * [te[t, j] == e].
    tok_lists = []
    wt_lists = []
    for e in range(E):
        sel = (te == e)                    # [N, topk]
        toks = np.nonzero(sel.any(axis=1))[0]
        wts = (ew[toks] * sel[toks]).sum(axis=1)
        tok_lists.append(toks)
        wt_lists.append(wts)
    counts = np.array([len(t) for t in tok_lists])

    cmax = int(counts.max())
    cw, nch = _chunking(min(max(cmax, 1), C_MAX))
    cap = cw * nch
    n_waves = max(1, -(-cmax // cap))

    key = (int(cw), int(nch), H, F, n_cores)
    if key not in _cache:
        _cache[key] = _build(*key)
    nc = _cache[key]

    # per-expert constant inputs (weights pre-tiled to contiguous SBUF strips)
    KT1, FT, FP, MT = H // P, F2 // P, F // P, H // P
    const_maps = []
    for e in range(E):
        w1e = np.asarray(w1[e], dtype=np.float32).astype(ml_dtypes.bfloat16)
        w1t = np.ascontiguousarray(
            w1e.reshape(KT1, P, FT, P).transpose(2, 1, 0, 3))
        w2e = np.asarray(w2[e], dtype=np.float32).astype(ml_dtypes.bfloat16)
        w2t = np.ascontiguousarray(
            w2e.reshape(FP, P, MT, P).transpose(2, 1, 0, 3))
        const_maps.append({
            "w1t": w1t,
            "b1t": np.ascontiguousarray(
                np.asarray(b1[e], dtype=np.float32).reshape(FT, P).T),
            "w2t": w2t,
            "b2t": np.ascontiguousarray(
                np.asarray(b2[e], dtype=np.float32).reshape(MT, P).T),
        })

    trace = os.environ.get("KERNEL_TRACE", "") == "1"
    out = np.zeros((N, H), dtype=np.float32)
    last_results = []
    for w in range(n_waves):
        in_maps = []
        for e in range(E):
            lo = w * cap
            toks = tok_lists[e][lo: lo + cap]
            xt = np.zeros((H, cap), dtype=ml_dtypes.bfloat16)
            if len(toks):
                xt[:, :len(toks)] = x[toks].T.astype(ml_dtypes.bfloat16)
            in_maps.append({"xt": xt, **const_maps[e]})
        tmpdir = None
        if trace:
            import shutil
            tmpdir = f"/tmp/moe_trace_w{w}"
            shutil.rmtree(tmpdir, ignore_errors=True)
            os.makedirs(tmpdir, exist_ok=True)
        res = bass_utils.run_bass_kernel_spmd(
            nc, in_maps, core_ids=list(range(n_cores)), trace=trace,
            tmpdir=tmpdir)
        last_results.append(res)
        if trace:
            last_exec_time_ns = res.exec_time_ns
        for e in range(E):
            lo = w * cap
            toks = tok_lists[e][lo: lo + cap]
            if len(toks):
                yt = res.results[e]["yt"]
                wts = wt_lists[e][lo: lo + cap]
                out[toks] += wts[:, None] * yt[:, :len(toks)].T

    return out.reshape(B, S, H).astype(np.float32)


# revision 28
# speedup vs baseline: 1.0164x; 1.0020x over previous
# Grouped-GEMM MoE (8 experts, top-2, SwiGLU) on 8 Trainium2 NeuronCores.
#
# Strategy (expert-parallel, host-side all-to-all):
#   - Host routes tokens to experts. Tokens whose top-2 picks the same
#     expert twice are DEDUPLICATED (y is identical for both slots; the
#     combine weight becomes ew1+ew2) — ~1/8 of tokens, so the per-expert
#     batch shrinks accordingly.
#   - Core e runs expert e's dense MLP feature-major: every matmul contracts
#     over the partition dim, the token dim always rides the free dim, so no
#     on-device transposes are needed:
#         H.T  = W1.T-tiles @ X.T      (bf16, K=2048)
#         actT = silu(Ha + b1a) * (Hb + b1b)   (ACT + DVE, fused bias)
#         Y.T  = W2.T-tiles @ actT     (bf16, K=2816)
#   - Host scatters Y rows back to tokens and does the weighted top-k
#     combine.
#
# Weights stream from HBM exactly once per call (pre-tiled host-side so each
# SBUF strip is one contiguous read); activations stay resident.

import os

import ml_dtypes
import numpy as np

import concourse.bacc as bacc
import concourse.mybir as mybir
import concourse.tile as tile
from concourse import bass_utils
from concourse.bass import ts

P = 128
C_MAX = 1536      # max token capacity per wave (SBUF-resident xt + act)

f32 = mybir.dt.float32
bf16 = mybir.dt.bfloat16
Silu = mybir.ActivationFunctionType.Silu
Identity = mybir.ActivationFunctionType.Identity
Alu = mybir.AluOpType

_cache = {}

# set by the most recent kernel() call when KERNEL_TRACE=1 (test harness use)
last_exec_time_ns = None
last_results = None


def _chunking(cmax):
    """Pick (chunk_width, n_chunks): CW <= 512 (PSUM bank), minimal padded
    capacity, CW large enough that weight loads stay hidden."""
    NCH = max(1, -(-cmax // 512))
    cw = max(256, ((-(-cmax // NCH)) + 3) // 4 * 4)
    return cw, NCH


def _build(CW, NCH, H, F, n_cores):
    """Build+schedule the per-core MLP program for token capacity CW*NCH."""
    C = CW * NCH
    F2 = 2 * F
    KT1 = H // P      # k-tiles of GEMM1 (16)
    FT = F2 // P      # f-tiles of W1    (44)
    FP = F // P       # f-tile pairs / k-tiles of GEMM2 (22)
    MT = H // P       # m-tiles of GEMM2 (16)

    nc = bacc.Bacc("TRN2", target_bir_lowering=False, debug=False,
                   num_devices=n_cores)

    xt_d = nc.dram_tensor("xt", (H, C), bf16, kind="ExternalInput").ap()
    # pre-tiled weights: w1t[ft, p, ko, fi] = w1[ko*P + p, ft*P + fi]
    w1_d = nc.dram_tensor("w1t", (FT, P, KT1, P), bf16, kind="ExternalInput").ap()
    # biases pre-tiled host-side to [P, n] so the DMA descriptors are
    # per-partition contiguous: b1t[p, ft] = b1[ft*P + p]
    b1_d = nc.dram_tensor("b1t", (P, FT), f32, kind="ExternalInput").ap()
    # w2t[mt, p, ko, mi] = w2[ko*P + p, mt*P + mi]  (bf16)
    w2_d = nc.dram_tensor("w2t", (MT, P, FP, P), bf16, kind="ExternalInput").ap()
    b2_d = nc.dram_tensor("b2t", (P, MT), f32, kind="ExternalInput").ap()
    yt_d = nc.dram_tensor("yt", (H, C), f32, kind="ExternalOutput").ap()

    xt_t = xt_d.rearrange("(ko p) c -> p ko c", p=P)    # [128, KT1, C]
    yt_t = yt_d.rearrange("(mt p) c -> p mt c", p=P)

    # Stagger chunk ch by LAG f-iterations behind chunk 0 so the startup
    # only waits on chunk 0's xt tiles + the first weight strips (the DMA
    # queues deliver xt chunk 1+ and later weights while PE is busy).
    # Smaller LAG at NCH>=3 keeps the w1 pair-pinning window (and so the
    # weight pool) small enough for SBUF.
    LAG = 4 if NCH <= 2 else 2
    order = []
    for step in range(FP + (NCH - 1) * LAG):
        for ch in range(NCH):
            f = step - ch * LAG
            if 0 <= f < FP:
                order.append((f, ch))

    with tile.TileContext(nc) as tc:
        with tc.tile_pool(name="persist", bufs=1) as persist, \
             tc.tile_pool(name="w1pool", bufs=2 * ((NCH - 1) * LAG + 3)) as w1pool, \
             tc.tile_pool(name="w2pool", bufs=2) as w2pool, \
             tc.tile_pool(name="evac", bufs=3) as evac, \
             tc.tile_pool(name="ps1", bufs=2, space="PSUM") as ps1, \
             tc.tile_pool(name="ps2", bufs=4, space="PSUM") as ps2:

            # xt tiles in groups of GK k-tiles per DMA (fewer serialized
            # trigger slots). Chunk 0 streams on the two otherwise-idle HWDGE
            # queues (sync + vector) at t=0; later chunks are only needed LAG
            # f-iterations in, so they queue up behind the first w1 strips on
            # the weight queue (see below).
            GK = 4
            NG = KT1 // GK
            xt_sb = {}
            xt_groups = {}
            for g in range(NG):
                t = persist.tile([P, GK, CW], bf16, tag=f"xtg_{g}_0")
                if g == 0:
                    # halve the very first transfer so the first matmul
                    # starts as early as possible
                    nc.sync.dma_start(t[:, :GK // 2],
                                      xt_t[:, :GK // 2, ts(0, CW)])
                    nc.sync.dma_start(t[:, GK // 2:],
                                      xt_t[:, ts(1, GK // 2), ts(0, CW)])
                else:
                    nc.sync.dma_start(t[:], xt_t[:, ts(g, GK), ts(0, CW)])
                xt_groups[g, 0] = t
                for j in range(GK):
                    xt_sb[g * GK + j, 0] = t[:, j]
            for ch in range(1, NCH):
                for g in range(NG):
                    t = persist.tile([P, GK, CW], bf16, tag=f"xtg_{g}_{ch}")
                    xt_groups[g, ch] = t
                    for j in range(GK):
                        xt_sb[g * GK + j, ch] = t[:, j]
            act_sb = persist.tile([P, FP, C], bf16)
            b1_sb = persist.tile([P, FT], f32)
            b2_sb = persist.tile([P, MT], f32)

            # ---- GEMM1 + SwiGLU, one f-tile pair (a, b halves) at a time
            with nc.named_scope("gemm1"):
                w1_sb = {}
                n_pairs = 0
                for fp, ch in order:
                    if fp not in w1_sb:
                        w1a = w1pool.tile([P, KT1, P], bf16, tag="w1s")
                        w1b = w1pool.tile([P, KT1, P], bf16, tag="w1s")
                        if n_pairs < 1:
                            # halve the first pair's strips so the first
                            # matmul chain starts as soon as k-tiles land;
                            # a/b interleaved to match the chain order below
                            hk = KT1 // 2
                            for q in range(2):
                                nc.scalar.dma_start(w1a[:, ts(q, hk)],
                                                    w1_d[fp][:, ts(q, hk)])
                                nc.scalar.dma_start(w1b[:, ts(q, hk)],
                                                    w1_d[FP + fp][:, ts(q, hk)])
                        else:
                            nc.scalar.dma_start(w1a[:], w1_d[fp])
                            nc.scalar.dma_start(w1b[:], w1_d[FP + fp])
                        w1_sb[fp] = (w1a, w1b)
                        n_pairs += 1
                        if n_pairs == 1:
                            # biases ride behind the first pair (first needed
                            # by the fp=0 SwiGLU, ~15us in)
                            nc.sync.dma_start(b1_sb[:], b1_d)
                            nc.sync.dma_start(b2_sb[:], b2_d)
                        if n_pairs == 2:
                            # xt chunks 1+ ride the weight queue behind the
                            # first two pairs (needed from step LAG onward)
                            for ch2 in range(1, NCH):
                                for g in range(NG):
                                    nc.scalar.dma_start(
                                        xt_groups[g, ch2][:],
                                        xt_t[:, ts(g, GK), ts(ch2, CW)])
                    w1a, w1b = w1_sb[fp]
                    pa = ps1.tile([P, CW], f32, tag="pa")
                    pb = ps1.tile([P, CW], f32, tag="pb")
                    if fp == 0 and ch == 0:
                        # startup: interleave the a/b chains per xt group so
                        # each arriving group feeds 2*GK matmuls and PE never
                        # outruns the DMA pipe
                        for g in range(NG):
                            for half, p_ in ((w1a, pa), (w1b, pb)):
                                for k in range(g * GK, (g + 1) * GK):
                                    nc.tensor.matmul(
                                        p_, half[:, k], xt_sb[k, ch][:],
                                        start=(k == 0), stop=(k == KT1 - 1))
                    else:
                        for k in range(KT1):
                            nc.tensor.matmul(pa, w1a[:, k],
                                             xt_sb[k, ch][:],
                                             start=(k == 0), stop=(k == KT1 - 1))
                        for k in range(KT1):
                            nc.tensor.matmul(pb, w1b[:, k],
                                             xt_sb[k, ch][:],
                                             start=(k == 0), stop=(k == KT1 - 1))
                    s = evac.tile([P, CW], f32, tag="silu")
                    nc.scalar.activation(s, pa, Silu,
                                         bias=b1_sb[:, fp:fp + 1])
                    # act = (pb + b1b) * silu(pa + b1a), cast bf16 on write
                    nc.vector.scalar_tensor_tensor(
                        act_sb[:, fp, ts(ch, CW)], pb,
                        b1_sb[:, FP + fp:FP + fp + 1], s,
                        Alu.add, Alu.mult)
                    if ch == NCH - 1:
                        del w1_sb[fp]

            # ---- GEMM2
            with nc.named_scope("gemm2"):
                for m in range(MT):
                    # same queue as w1 so the w2 prefetch queues up BEHIND the
                    # startup-critical w1 strips instead of competing at t=0
                    w2s = w2pool.tile([P, FP, P], bf16, tag="w2s")
                    nc.scalar.dma_start(w2s[:], w2_d[m])
                    for ch in range(NCH):
                        py = ps2.tile([P, CW], f32, tag="py")
                        for k in range(FP):
                            nc.tensor.matmul(py, w2s[:, k],
                                             act_sb[:, k, ts(ch, CW)],
                                             start=(k == 0), stop=(k == FP - 1))
                        y = evac.tile([P, CW], f32, tag="y")
                        nc.scalar.activation(y, py, Identity,
                                             bias=b2_sb[:, m:m + 1])
                        nc.sync.dma_start(yt_t[:, m, ts(ch, CW)], y[:])

    nc.compile()
    return nc


def kernel(hidden_states, expert_weights, w1, b1, w2, b2, top_experts):
    global last_exec_time_ns, last_results

    hidden_states = np.asarray(hidden_states)
    B, S, H = hidden_states.shape
    E, _, F2 = np.asarray(w1).shape
    F = F2 // 2
    topk = np.asarray(top_experts).shape[-1]
    N = B * S
    n_cores = 8
    assert E == n_cores, f"kernel assumes one expert per core, got E={E}"

    x = np.ascontiguousarray(hidden_states.reshape(N, H).astype(np.float32))
    te = np.asarray(top_experts).reshape(N, topk).astype(np.int64)
    ew = np.asarray(expert_weights).reshape(N, topk).astype(np.float32)

    # Dedup routing: token t needs expert e once, with combined weight
    # sum_j ew[t, j] * [te[t, j] == e].
    tok_lists = []
    wt_lists = []
    for e in range(E):
        sel = (te == e)                    # [N, topk]
        toks = np.nonzero(sel.any(axis=1))[0]
        wts = (ew[toks] * sel[toks]).sum(axis=1)
        tok_lists.append(toks)
        wt_lists.append(wts)
    counts = np.array([len(t) for t in tok_lists])

    cmax = int(counts.max())
    cw, nch = _chunking(min(max(cmax, 1), C_MAX))
    cap = cw * nch
    n_waves = max(1, -(-cmax // cap))

    key = (int(cw), int(nch), H, F, n_cores)
    if key not in _cache:
        _cache[key] = _build(*key)
    nc = _cache[key]

    # per-expert constant inputs (weights pre-tiled to contiguous SBUF strips)
    KT1, FT, FP, MT = H // P, F2 // P, F // P, H // P
    const_maps = []
    for e in range(E):
        w1e = np.asarray(w1[e], dtype=np.float32).astype(ml_dtypes.bfloat16)
        w1t = np.ascontiguousarray(
            w1e.reshape(KT1, P, FT, P).transpose(2, 1, 0, 3))
        w2e = np.asarray(w2[e], dtype=np.float32).astype(ml_dtypes.bfloat16)
        w2t = np.ascontiguousarray(
            w2e.reshape(FP, P, MT, P).transpose(2, 1, 0, 3))
        const_maps.append({
            "w1t": w1t,
            "b1t": np.ascontiguousarray(
                np.asarray(b1[e], dtype=np.float32).reshape(FT, P).T),
            "w2t": w2t,
            "b2t": np.ascontiguousarray(
                np.asarray(b2[e], dtype=np.float32).reshape(MT, P).T),
        })

    trace = os.environ.get("KERNEL_TRACE", "") == "1"
    out = np.zeros((N, H), dtype=np.float32)
    last_results = []
    for w in range(n_waves):
        in_maps = []
        for e in range(E):
            lo = w * cap
            toks = tok_lists[e][lo: lo + cap]
            xt = np.zeros((H, cap), dtype=ml_dtypes.bfloat16)
            if len(toks):
                xt[:, :len(toks)] = x[toks].T.astype(ml_dtypes.bfloat16)
            in_maps.append({"xt": xt, **const_maps[e]})
        tmpdir = None
        if trace:
            import shutil
            tmpdir = f"/tmp/moe_trace_w{w}"
            shutil.rmtree(tmpdir, ignore_errors=True)
            os.makedirs(tmpdir, exist_ok=True)
        res = bass_utils.run_bass_kernel_spmd(
            nc, in_maps, core_ids=list(range(n_cores)), trace=trace,
            tmpdir=tmpdir)
        last_results.append(res)
        if trace:
            last_exec_time_ns = res.exec_time_ns
        for e in range(E):
            lo = w * cap
            toks = tok_lists[e][lo: lo + cap]
            if len(toks):
                yt = res.results[e]["yt"]
                wts = wt_lists[e][lo: lo + cap]
                out[toks] += wts[:, None] * yt[:, :len(toks)].T

    return out.reshape(B, S, H).astype(np.float32)


# revision 31
# speedup vs baseline: 1.0167x; 1.0003x over previous
# Grouped-GEMM MoE (8 experts, top-2, SwiGLU) on 8 Trainium2 NeuronCores.
#
# Strategy (expert-parallel, host-side all-to-all):
#   - Host routes tokens to experts. Tokens whose top-2 picks the same
#     expert twice are DEDUPLICATED (y is identical for both slots; the
#     combine weight becomes ew1+ew2) — ~1/8 of tokens, so the per-expert
#     batch shrinks accordingly.
#   - Core e runs expert e's dense MLP feature-major: every matmul contracts
#     over the partition dim, the token dim always rides the free dim, so no
#     on-device transposes are needed:
#         H.T  = W1.T-tiles @ X.T      (bf16, K=2048)
#         actT = silu(Ha + b1a) * (Hb + b1b)   (ACT + DVE, fused bias)
#         Y.T  = W2.T-tiles @ actT     (bf16, K=2816)
#   - Host scatters Y rows back to tokens and does the weighted top-k
#     combine.
#
# Weights stream from HBM exactly once per call (pre-tiled host-side so each
# SBUF strip is one contiguous read); activations stay resident.

import os

import ml_dtypes
import numpy as np

import concourse.bacc as bacc
import concourse.mybir as mybir
import concourse.tile as tile
from concourse import bass_utils
from concourse.bass import ts

P = 128
C_MAX = 1536      # max token capacity per wave (SBUF-resident xt + act)

f32 = mybir.dt.float32
bf16 = mybir.dt.bfloat16
Silu = mybir.ActivationFunctionType.Silu
Identity = mybir.ActivationFunctionType.Identity
Alu = mybir.AluOpType

_cache = {}

# set by the most recent kernel() call when KERNEL_TRACE=1 (test harness use)
last_exec_time_ns = None
last_results = None


def _chunking(cmax):
    """Pick (chunk_width, n_chunks): CW <= 512 (PSUM bank), minimal padded
    capacity, CW large enough that weight loads stay hidden."""
    NCH = max(1, -(-cmax // 512))
    cw = max(256, ((-(-cmax // NCH)) + 1) // 2 * 2)
    return cw, NCH


def _build(CW, NCH, H, F, n_cores):
    """Build+schedule the per-core MLP program for token capacity CW*NCH."""
    C = CW * NCH
    F2 = 2 * F
    KT1 = H // P      # k-tiles of GEMM1 (16)
    FT = F2 // P      # f-tiles of W1    (44)
    FP = F // P       # f-tile pairs / k-tiles of GEMM2 (22)
    MT = H // P       # m-tiles of GEMM2 (16)

    nc = bacc.Bacc("TRN2", target_bir_lowering=False, debug=False,
                   num_devices=n_cores)

    xt_d = nc.dram_tensor("xt", (H, C), bf16, kind="ExternalInput").ap()
    # pre-tiled weights: w1t[ft, p, ko, fi] = w1[ko*P + p, ft*P + fi]
    w1_d = nc.dram_tensor("w1t", (FT, P, KT1, P), bf16, kind="ExternalInput").ap()
    # biases pre-tiled host-side to [P, n] so the DMA descriptors are
    # per-partition contiguous: b1t[p, ft] = b1[ft*P + p]
    b1_d = nc.dram_tensor("b1t", (P, FT), f32, kind="ExternalInput").ap()
    # w2t[mt, p, ko, mi] = w2[ko*P + p, mt*P + mi]  (bf16)
    w2_d = nc.dram_tensor("w2t", (MT, P, FP, P), bf16, kind="ExternalInput").ap()
    b2_d = nc.dram_tensor("b2t", (P, MT), f32, kind="ExternalInput").ap()
    yt_d = nc.dram_tensor("yt", (H, C), f32, kind="ExternalOutput").ap()

    xt_t = xt_d.rearrange("(ko p) c -> p ko c", p=P)    # [128, KT1, C]
    yt_t = yt_d.rearrange("(mt p) c -> p mt c", p=P)

    # Stagger chunk ch by LAG f-iterations behind chunk 0 so the startup
    # only waits on chunk 0's xt tiles + the first weight strips (the DMA
    # queues deliver xt chunk 1+ and later weights while PE is busy).
    # Smaller LAG at NCH>=3 keeps the w1 pair-pinning window (and so the
    # weight pool) small enough for SBUF.
    LAG = 4 if NCH <= 2 else 2
    order = []
    for step in range(FP + (NCH - 1) * LAG):
        for ch in range(NCH):
            f = step - ch * LAG
            if 0 <= f < FP:
                order.append((f, ch))

    with tile.TileContext(nc) as tc:
        with tc.tile_pool(name="persist", bufs=1) as persist, \
             tc.tile_pool(name="w1pool", bufs=2 * ((NCH - 1) * LAG + 3)) as w1pool, \
             tc.tile_pool(name="w2pool", bufs=2) as w2pool, \
             tc.tile_pool(name="evac", bufs=3) as evac, \
             tc.tile_pool(name="ps1", bufs=2, space="PSUM") as ps1, \
             tc.tile_pool(name="ps2", bufs=4, space="PSUM") as ps2:

            # xt tiles in groups of GK k-tiles per DMA (fewer serialized
            # trigger slots). Chunk 0 streams on the two otherwise-idle HWDGE
            # queues (sync + vector) at t=0; later chunks are only needed LAG
            # f-iterations in, so they queue up behind the first w1 strips on
            # the weight queue (see below).
            GK = 4
            NG = KT1 // GK
            xt_sb = {}
            xt_groups = {}
            for g in range(NG):
                t = persist.tile([P, GK, CW], bf16, tag=f"xtg_{g}_0")
                if g == 0:
                    # minimal first piece so the first matmul starts as
                    # early as possible
                    nc.sync.dma_start(t[:, :1], xt_t[:, :1, ts(0, CW)])
                    nc.sync.dma_start(t[:, 1:], xt_t[:, 1:GK, ts(0, CW)])
                else:
                    nc.sync.dma_start(t[:], xt_t[:, ts(g, GK), ts(0, CW)])
                xt_groups[g, 0] = t
                for j in range(GK):
                    xt_sb[g * GK + j, 0] = t[:, j]
            for ch in range(1, NCH):
                for g in range(NG):
                    t = persist.tile([P, GK, CW], bf16, tag=f"xtg_{g}_{ch}")
                    xt_groups[g, ch] = t
                    for j in range(GK):
                        xt_sb[g * GK + j, ch] = t[:, j]
            act_sb = persist.tile([P, FP, C], bf16)
            b1_sb = persist.tile([P, FT], f32)
            b2_sb = persist.tile([P, MT], f32)

            # ---- GEMM1 + SwiGLU, one f-tile pair (a, b halves) at a time
            with nc.named_scope("gemm1"):
                w1_sb = {}
                n_pairs = 0
                for fp, ch in order:
                    if fp not in w1_sb:
                        w1a = w1pool.tile([P, KT1, P], bf16, tag="w1s")
                        w1b = w1pool.tile([P, KT1, P], bf16, tag="w1s")
                        if n_pairs < 1:
                            # small leading piece of the first pair's strips
                            # so the first matmul chain starts as soon as
                            # k-tiles land; a/b interleaved to match the
                            # chain order below
                            for lo, hi in ((0, GK), (GK, KT1)):
                                nc.scalar.dma_start(w1a[:, lo:hi],
                                                    w1_d[fp][:, lo:hi])
                                nc.scalar.dma_start(w1b[:, lo:hi],
                                                    w1_d[FP + fp][:, lo:hi])
                        else:
                            nc.scalar.dma_start(w1a[:], w1_d[fp])
                            nc.scalar.dma_start(w1b[:], w1_d[FP + fp])
                        w1_sb[fp] = (w1a, w1b)
                        n_pairs += 1
                        if n_pairs == 1:
                            # biases ride behind the first pair (first needed
                            # by the fp=0 SwiGLU, ~15us in)
                            nc.sync.dma_start(b1_sb[:], b1_d)
                            nc.sync.dma_start(b2_sb[:], b2_d)
                        if n_pairs == 2:
                            # xt chunks 1+ ride the weight queue behind the
                            # first two pairs (needed from step LAG onward)
                            for ch2 in range(1, NCH):
                                for g in range(NG):
                                    nc.scalar.dma_start(
                                        xt_groups[g, ch2][:],
                                        xt_t[:, ts(g, GK), ts(ch2, CW)])
                    w1a, w1b = w1_sb[fp]
                    pa = ps1.tile([P, CW], f32, tag="pa")
                    pb = ps1.tile([P, CW], f32, tag="pb")
                    if fp == 0 and ch == 0:
                        # startup: interleave the a/b chains per xt group so
                        # each arriving group feeds 2*GK matmuls and PE never
                        # outruns the DMA pipe
                        for g in range(NG):
                            for half, p_ in ((w1a, pa), (w1b, pb)):
                                for k in range(g * GK, (g + 1) * GK):
                                    nc.tensor.matmul(
                                        p_, half[:, k], xt_sb[k, ch][:],
                                        start=(k == 0), stop=(k == KT1 - 1))
                    else:
                        for k in range(KT1):
                            nc.tensor.matmul(pa, w1a[:, k],
                                             xt_sb[k, ch][:],
                                             start=(k == 0), stop=(k == KT1 - 1))
                        for k in range(KT1):
                            nc.tensor.matmul(pb, w1b[:, k],
                                             xt_sb[k, ch][:],
                                             start=(k == 0), stop=(k == KT1 - 1))
                    s = evac.tile([P, CW], f32, tag="silu")
                    nc.scalar.activation(s, pa, Silu,
                                         bias=b1_sb[:, fp:fp + 1])
                    # act = (pb + b1b) * silu(pa + b1a), cast bf16 on write
                    nc.vector.scalar_tensor_tensor(
                        act_sb[:, fp, ts(ch, CW)], pb,
                        b1_sb[:, FP + fp:FP + fp + 1], s,
                        Alu.add, Alu.mult)
                    if ch == NCH - 1:
                        del w1_sb[fp]

            # ---- GEMM2
            with nc.named_scope("gemm2"):
                for m in range(MT):
                    # same queue as w1 so the w2 prefetch queues up BEHIND the
                    # startup-critical w1 strips instead of competing at t=0
                    w2s = w2pool.tile([P, FP, P], bf16, tag="w2s")
                    nc.scalar.dma_start(w2s[:], w2_d[m])
                    for ch in range(NCH):
                        py = ps2.tile([P, CW], f32, tag="py")
                        for k in range(FP):
                            nc.tensor.matmul(py, w2s[:, k],
                                             act_sb[:, k, ts(ch, CW)],
                                             start=(k == 0), stop=(k == FP - 1))
                        y = evac.tile([P, CW], f32, tag="y")
                        nc.scalar.activation(y, py, Identity,
                                             bias=b2_sb[:, m:m + 1])
                        nc.sync.dma_start(yt_t[:, m, ts(ch, CW)], y[:])

    nc.compile()
    return nc


def kernel(hidden_states, expert_weights, w1, b1, w2, b2, top_experts):
    global last_exec_time_ns, last_results

    hidden_states = np.asarray(hidden_states)
    B, S, H = hidden_states.shape
    E, _, F2 = np.asarray(w1).shape
    F = F2 // 2
    topk = np.asarray(top_experts).shape[-1]
    N = B * S
    n_cores = 8
    assert E == n_cores, f"kernel assumes one expert per core, got E={E}"

    x = np.ascontiguousarray(hidden_states.reshape(N, H).astype(np.float32))
    te = np.asarray(top_experts).reshape(N, topk).astype(np.int64)
    ew = np.asarray(expert_weights).reshape(N, topk).astype(np.float32)

    # Dedup routing: token t needs expert e once, with combined weight
    # sum_j ew[t, j] * [te[t, j] == e].
    tok_lists = []
    wt_lists = []
    for e in range(E):
        sel = (te == e)                    # [N, topk]
        toks = np.nonzero(sel.any(axis=1))[0]
        wts = (ew[toks] * sel[toks]).sum(axis=1)
        tok_lists.append(toks)
        wt_lists.append(wts)
    counts = np.array([len(t) for t in tok_lists])

    cmax = int(counts.max())
    cw, nch = _chunking(min(max(cmax, 1), C_MAX))
    cap = cw * nch
    n_waves = max(1, -(-cmax // cap))

    key = (int(cw), int(nch), H, F, n_cores)
    if key not in _cache:
        _cache[key] = _build(*key)
    nc = _cache[key]

    # per-expert constant inputs (weights pre-tiled to contiguous SBUF strips)
    KT1, FT, FP, MT = H // P, F2 // P, F // P, H // P
    const_maps = []
    for e in range(E):
        w1e = np.asarray(w1[e], dtype=np.float32).astype(ml_dtypes.bfloat16)
        w1t = np.ascontiguousarray(
            w1e.reshape(KT1, P, FT, P).transpose(2, 1, 0, 3))
        w2e = np.asarray(w2[e], dtype=np.float32).astype(ml_dtypes.bfloat16)
        w2t = np.ascontiguousarray(
            w2e.reshape(FP, P, MT, P).transpose(2, 1, 0, 3))
        const_maps.append({
            "w1t": w1t,
            "b1t": np.ascontiguousarray(
                np.asarray(b1[e], dtype=np.float32).reshape(FT, P).T),
            "w2t": w2t,
            "b2t": np.ascontiguousarray(
                np.asarray(b2[e], dtype=np.float32).reshape(MT, P).T),
        })

    trace = os.environ.get("KERNEL_TRACE", "") == "1"
    out = np.zeros((N, H), dtype=np.float32)
    last_results = []
    for w in range(n_waves):
        in_maps = []
        for e in range(E):
            lo = w * cap
            toks = tok_lists[e][lo: lo + cap]
            xt = np.zeros((H, cap), dtype=ml_dtypes.bfloat16)
            if len(toks):
                xt[:, :len(toks)] = x[toks].T.astype(ml_dtypes.bfloat16)
            in_maps.append({"xt": xt, **const_maps[e]})
        tmpdir = None
        if trace:
            import shutil
            tmpdir = f"/tmp/moe_trace_w{w}"
            shutil.rmtree(tmpdir, ignore_errors=True)
            os.makedirs(tmpdir, exist_ok=True)
        res = bass_utils.run_bass_kernel_spmd(
            nc, in_maps, core_ids=list(range(n_cores)), trace=trace,
            tmpdir=tmpdir)
        last_results.append(res)
        if trace:
            last_exec_time_ns = res.exec_time_ns
        for e in range(E):
            lo = w * cap
            toks = tok_lists[e][lo: lo + cap]
            if len(toks):
                yt = res.results[e]["yt"]
                wts = wt_lists[e][lo: lo + cap]
                out[toks] += wts[:, None] * yt[:, :len(toks)].T

    return out.reshape(B, S, H).astype(np.float32)


# revision 34
# speedup vs baseline: 1.0177x; 1.0010x over previous
# Grouped-GEMM MoE (8 experts, top-2, SwiGLU) on 8 Trainium2 NeuronCores.
#
# Strategy (expert-parallel, host-side all-to-all):
#   - Host routes tokens to experts. Tokens whose top-2 picks the same
#     expert twice are DEDUPLICATED (y is identical for both slots; the
#     combine weight becomes ew1+ew2) — ~1/8 of tokens, so the per-expert
#     batch shrinks accordingly.
#   - Core e runs expert e's dense MLP feature-major: every matmul contracts
#     over the partition dim, the token dim always rides the free dim, so no
#     on-device transposes are needed:
#         H.T  = W1.T-tiles @ X.T      (bf16, K=2048)
#         actT = silu(Ha + b1a) * (Hb + b1b)   (ACT + DVE, fused bias)
#         Y.T  = W2.T-tiles @ actT     (bf16, K=2816)
#   - Host scatters Y rows back to tokens and does the weighted top-k
#     combine.
#
# Weights stream from HBM exactly once per call (pre-tiled host-side so each
# SBUF strip is one contiguous read); activations stay resident.

import os

import ml_dtypes
import numpy as np

import concourse.bacc as bacc
import concourse.mybir as mybir
import concourse.tile as tile
from concourse import bass_utils
from concourse.bass import ts

P = 128
C_MAX = 1536      # max token capacity per wave (SBUF-resident xt + act)

f32 = mybir.dt.float32
bf16 = mybir.dt.bfloat16
Silu = mybir.ActivationFunctionType.Silu
Identity = mybir.ActivationFunctionType.Identity
Alu = mybir.AluOpType

_cache = {}

# set by the most recent kernel() call when KERNEL_TRACE=1 (test harness use)
last_exec_time_ns = None
last_results = None


def _chunking(cmax):
    """Pick (chunk_width, n_chunks): CW <= 512 (PSUM bank), minimal padded
    capacity, CW large enough that weight loads stay hidden."""
    NCH = max(1, -(-cmax // 512))
    cw = max(256, ((-(-cmax // NCH)) + 1) // 2 * 2)
    return cw, NCH


def _build(CW, NCH, H, F, n_cores):
    """Build+schedule the per-core MLP program for token capacity CW*NCH."""
    C = CW * NCH
    F2 = 2 * F
    KT1 = H // P      # k-tiles of GEMM1 (16)
    FT = F2 // P      # f-tiles of W1    (44)
    FP = F // P       # f-tile pairs / k-tiles of GEMM2 (22)
    MT = H // P       # m-tiles of GEMM2 (16)

    nc = bacc.Bacc("TRN2", target_bir_lowering=False, debug=False,
                   num_devices=n_cores)

    xt_d = nc.dram_tensor("xt", (H, C), bf16, kind="ExternalInput").ap()
    # pre-tiled weights: w1t[ft, p, ko, fi] = w1[ko*P + p, ft*P + fi]
    w1_d = nc.dram_tensor("w1t", (FT, P, KT1, P), bf16, kind="ExternalInput").ap()
    # biases pre-tiled host-side to [P, n] so the DMA descriptors are
    # per-partition contiguous: b1t[p, ft] = b1[ft*P + p]
    b1_d = nc.dram_tensor("b1t", (P, FT), f32, kind="ExternalInput").ap()
    # w2t[mt, p, ko, mi] = w2[ko*P + p, mt*P + mi]  (bf16)
    w2_d = nc.dram_tensor("w2t", (MT, P, FP, P), bf16, kind="ExternalInput").ap()
    b2_d = nc.dram_tensor("b2t", (P, MT), f32, kind="ExternalInput").ap()
    yt_d = nc.dram_tensor("yt", (H, C), f32, kind="ExternalOutput").ap()

    xt_t = xt_d.rearrange("(ko p) c -> p ko c", p=P)    # [128, KT1, C]
    yt_t = yt_d.rearrange("(mt p) c -> p mt c", p=P)

    # Stagger chunk ch by LAG f-iterations behind chunk 0 so the startup
    # only waits on chunk 0's xt tiles + the first weight strips (the DMA
    # queues deliver xt chunk 1+ and later weights while PE is busy).
    # Smaller LAG at NCH>=3 keeps the w1 pair-pinning window (and so the
    # weight pool) small enough for SBUF.
    LAG = 4 if NCH <= 2 else 2
    order = []
    for step in range(FP + (NCH - 1) * LAG):
        for ch in range(NCH):
            f = step - ch * LAG
            if 0 <= f < FP:
                order.append((f, ch))

    with tile.TileContext(nc) as tc:
        with tc.tile_pool(name="persist", bufs=1) as persist, \
             tc.tile_pool(name="w1pool", bufs=2 * ((NCH - 1) * LAG + 3)) as w1pool, \
             tc.tile_pool(name="w2pool", bufs=2) as w2pool, \
             tc.tile_pool(name="evac", bufs=3) as evac, \
             tc.tile_pool(name="ps1", bufs=2, space="PSUM") as ps1, \
             tc.tile_pool(name="ps2", bufs=4, space="PSUM") as ps2:

            # xt tiles in groups of GK k-tiles per DMA (fewer serialized
            # trigger slots). Chunk 0 streams on the two otherwise-idle HWDGE
            # queues (sync + vector) at t=0; later chunks are only needed LAG
            # f-iterations in, so they queue up behind the first w1 strips on
            # the weight queue (see below).
            GK = 4
            NG = KT1 // GK
            xt_sb = {}
            xt_groups = {}
            for g in range(NG):
                t = persist.tile([P, GK, CW], bf16, tag=f"xtg_{g}_0")
                if g == 0:
                    # halved first piece so the first matmul chain starts
                    # early without starving on k=1
                    h2 = GK // 2
                    nc.sync.dma_start(t[:, :h2], xt_t[:, :h2, ts(0, CW)])
                    nc.sync.dma_start(t[:, h2:], xt_t[:, h2:GK, ts(0, CW)])
                else:
                    nc.sync.dma_start(t[:], xt_t[:, ts(g, GK), ts(0, CW)])
                xt_groups[g, 0] = t
                for j in range(GK):
                    xt_sb[g * GK + j, 0] = t[:, j]
            for ch in range(1, NCH):
                for g in range(NG):
                    t = persist.tile([P, GK, CW], bf16, tag=f"xtg_{g}_{ch}")
                    xt_groups[g, ch] = t
                    for j in range(GK):
                        xt_sb[g * GK + j, ch] = t[:, j]
            act_sb = persist.tile([P, FP, C], bf16)
            b1_sb = persist.tile([P, FT], f32)
            b2_sb = persist.tile([P, MT], f32)

            # ---- GEMM1 + SwiGLU, one f-tile pair (a, b halves) at a time
            with nc.named_scope("gemm1"):
                w1_sb = {}
                n_pairs = 0
                for fp, ch in order:
                    if fp not in w1_sb:
                        w1a = w1pool.tile([P, KT1, P], bf16, tag="w1s")
                        w1b = w1pool.tile([P, KT1, P], bf16, tag="w1s")
                        if n_pairs < 1:
                            # small leading piece of the first pair's strips
                            # so the first matmul chain starts as soon as
                            # k-tiles land; a/b interleaved to match the
                            # chain order below
                            for lo, hi in ((0, GK), (GK, KT1)):
                                nc.scalar.dma_start(w1a[:, lo:hi],
                                                    w1_d[fp][:, lo:hi])
                                nc.scalar.dma_start(w1b[:, lo:hi],
                                                    w1_d[FP + fp][:, lo:hi])
                        else:
                            nc.scalar.dma_start(w1a[:], w1_d[fp])
                            nc.scalar.dma_start(w1b[:], w1_d[FP + fp])
                        w1_sb[fp] = (w1a, w1b)
                        n_pairs += 1
                        if n_pairs == 1:
                            # biases ride behind the first pair (first needed
                            # by the fp=0 SwiGLU, ~15us in)
                            nc.sync.dma_start(b1_sb[:], b1_d)
                            nc.sync.dma_start(b2_sb[:], b2_d)
                        if n_pairs == 2:
                            # xt chunks 1+ ride the weight queue behind the
                            # first two pairs (needed from step LAG onward)
                            for ch2 in range(1, NCH):
                                for g in range(NG):
                                    nc.scalar.dma_start(
                                        xt_groups[g, ch2][:],
                                        xt_t[:, ts(g, GK), ts(ch2, CW)])
                    w1a, w1b = w1_sb[fp]
                    pa = ps1.tile([P, CW], f32, tag="pa")
                    pb = ps1.tile([P, CW], f32, tag="pb")
                    if fp == 0 and ch == 0:
                        # startup: interleave the a/b chains per xt group so
                        # each arriving group feeds 2*GK matmuls and PE never
                        # outruns the DMA pipe
                        for g in range(NG):
                            for half, p_ in ((w1a, pa), (w1b, pb)):
                                for k in range(g * GK, (g + 1) * GK):
                                    nc.tensor.matmul(
                                        p_, half[:, k], xt_sb[k, ch][:],
                                        start=(k == 0), stop=(k == KT1 - 1))
                    else:
                        for k in range(KT1):
                            nc.tensor.matmul(pa, w1a[:, k],
                                             xt_sb[k, ch][:],
                                             start=(k == 0), stop=(k == KT1 - 1))
                        for k in range(KT1):
                            nc.tensor.matmul(pb, w1b[:, k],
                                             xt_sb[k, ch][:],
                                             start=(k == 0), stop=(k == KT1 - 1))
                    s = evac.tile([P, CW], f32, tag="silu")
                    nc.scalar.activation(s, pa, Silu,
                                         bias=b1_sb[:, fp:fp + 1])
                    # act = (pb + b1b) * silu(pa + b1a), cast bf16 on write
                    nc.vector.scalar_tensor_tensor(
                        act_sb[:, fp, ts(ch, CW)], pb,
                        b1_sb[:, FP + fp:FP + fp + 1], s,
                        Alu.add, Alu.mult)
                    if ch == NCH - 1:
                        del w1_sb[fp]

            # ---- GEMM2
            with nc.named_scope("gemm2"):
                for m in range(MT):
                    # same queue as w1 so the w2 prefetch queues up BEHIND the
                    # startup-critical w1 strips instead of competing at t=0
                    w2s = w2pool.tile([P, FP, P], bf16, tag="w2s")
                    nc.scalar.dma_start(w2s[:], w2_d[m])
                    for ch in range(NCH):
                        if m == MT - 1 and ch == NCH - 1:
                            # split the very last chunk into column halves so
                            # the first half's evac+DMA hides under the second
                            # half's matmuls, shortening the kernel tail
                            hw_ = CW // 2
                            for piece, (lo, w_) in enumerate(
                                    ((0, hw_), (hw_, CW - hw_))):
                                py = ps2.tile([P, w_], f32, tag="py")
                                for k in range(FP):
                                    nc.tensor.matmul(
                                        py, w2s[:, k],
                                        act_sb[:, k,
                                               ch * CW + lo:ch * CW + lo + w_],
                                        start=(k == 0), stop=(k == FP - 1))
                                y = evac.tile([P, w_], f32, tag="y")
                                nc.scalar.activation(y, py, Identity,
                                                     bias=b2_sb[:, m:m + 1])
                                eng = nc.sync if piece == 0 else nc.scalar
                                eng.dma_start(
                                    yt_t[:, m, ch * CW + lo:ch * CW + lo + w_],
                                    y[:])
                            continue
                        py = ps2.tile([P, CW], f32, tag="py")
                        for k in range(FP):
                            nc.tensor.matmul(py, w2s[:, k],
                                             act_sb[:, k, ts(ch, CW)],
                                             start=(k == 0), stop=(k == FP - 1))
                        y = evac.tile([P, CW], f32, tag="y")
                        nc.scalar.activation(y, py, Identity,
                                             bias=b2_sb[:, m:m + 1])
                        nc.sync.dma_start(yt_t[:, m, ts(ch, CW)], y[:])

    nc.compile()
    return nc


def kernel(hidden_states, expert_weights, w1, b1, w2, b2, top_experts):
    global last_exec_time_ns, last_results

    hidden_states = np.asarray(hidden_states)
    B, S, H = hidden_states.shape
    E, _, F2 = np.asarray(w1).shape
    F = F2 // 2
    topk = np.asarray(top_experts).shape[-1]
    N = B * S
    n_cores = 8
    assert E == n_cores, f"kernel assumes one expert per core, got E={E}"

    x = np.ascontiguousarray(hidden_states.reshape(N, H).astype(np.float32))
    te = np.asarray(top_experts).reshape(N, topk).astype(np.int64)
    ew = np.asarray(expert_weights).reshape(N, topk).astype(np.float32)

    # Dedup routing: token t needs expert e once, with combined weight
    # sum_j ew[t, j] * [te[t, j] == e].
    tok_lists = []
    wt_lists = []
    for e in range(E):
        sel = (te == e)                    # [N, topk]
        toks = np.nonzero(sel.any(axis=1))[0]
        wts = (ew[toks] * sel[toks]).sum(axis=1)
        tok_lists.append(toks)
        wt_lists.append(wts)
    counts = np.array([len(t) for t in tok_lists])

    cmax = int(counts.max())
    cw, nch = _chunking(min(max(cmax, 1), C_MAX))
    cap = cw * nch
    n_waves = max(1, -(-cmax // cap))

    key = (int(cw), int(nch), H, F, n_cores)
    if key not in _cache:
        _cache[key] = _build(*key)
    nc = _cache[key]

    # per-expert constant inputs (weights pre-tiled to contiguous SBUF strips)
    KT1, FT, FP, MT = H // P, F2 // P, F // P, H // P
    const_maps = []
    for e in range(E):
        w1e = np.asarray(w1[e], dtype=np.float32).astype(ml_dtypes.bfloat16)
        w1t = np.ascontiguousarray(
            w1e.reshape(KT1, P, FT, P).transpose(2, 1, 0, 3))
        w2e = np.asarray(w2[e], dtype=np.float32).astype(ml_dtypes.bfloat16)
        w2t = np.ascontiguousarray(
            w2e.reshape(FP, P, MT, P).transpose(2, 1, 0, 3))
        const_maps.append({
            "w1t": w1t,
            "b1t": np.ascontiguousarray(
                np.asarray(b1[e], dtype=np.float32).reshape(FT, P).T),
            "w2t": w2t,
            "b2t": np.ascontiguousarray(
                np.asarray(b2[e], dtype=np.float32).reshape(MT, P).T),
        })

    trace = os.environ.get("KERNEL_TRACE", "") == "1"
    out = np.zeros((N, H), dtype=np.float32)
    last_results = []
    for w in range(n_waves):
        in_maps = []
        for e in range(E):
            lo = w * cap
            toks = tok_lists[e][lo: lo + cap]
            xt = np.zeros((H, cap), dtype=ml_dtypes.bfloat16)
            if len(toks):
                xt[:, :len(toks)] = x[toks].T.astype(ml_dtypes.bfloat16)
            in_maps.append({"xt": xt, **const_maps[e]})
        tmpdir = None
        if trace:
            import shutil
            tmpdir = f"/tmp/moe_trace_w{w}"
            shutil.rmtree(tmpdir, ignore_errors=True)
            os.makedirs(tmpdir, exist_ok=True)
        res = bass_utils.run_bass_kernel_spmd(
            nc, in_maps, core_ids=list(range(n_cores)), trace=trace,
            tmpdir=tmpdir)
        last_results.append(res)
        if trace:
            last_exec_time_ns = res.exec_time_ns
        for e in range(E):
            lo = w * cap
            toks = tok_lists[e][lo: lo + cap]
            if len(toks):
                yt = res.results[e]["yt"]
                wts = wt_lists[e][lo: lo + cap]
                out[toks] += wts[:, None] * yt[:, :len(toks)].T

    return out.reshape(B, S, H).astype(np.float32)


# revision 36
# speedup vs baseline: 1.0197x; 1.0020x over previous
# Grouped-GEMM MoE (8 experts, top-2, SwiGLU) on 8 Trainium2 NeuronCores.
#
# Strategy (expert-parallel, host-side all-to-all):
#   - Host routes tokens to experts. Tokens whose top-2 picks the same
#     expert twice are DEDUPLICATED (y is identical for both slots; the
#     combine weight becomes ew1+ew2) — ~1/8 of tokens, so the per-expert
#     batch shrinks accordingly.
#   - Core e runs expert e's dense MLP feature-major: every matmul contracts
#     over the partition dim, the token dim always rides the free dim, so no
#     on-device transposes are needed:
#         H.T  = W1.T-tiles @ X.T      (bf16, K=2048)
#         actT = silu(Ha + b1a) * (Hb + b1b)   (ACT + DVE, fused bias)
#         Y.T  = W2.T-tiles @ actT     (bf16, K=2816)
#   - Host scatters Y rows back to tokens and does the weighted top-k
#     combine.
#
# Weights stream from HBM exactly once per call (pre-tiled host-side so each
# SBUF strip is one contiguous read); activations stay resident.

import os

import ml_dtypes
import numpy as np

import concourse.bacc as bacc
import concourse.mybir as mybir
import concourse.tile as tile
from concourse import bass_utils
from concourse.bass import ts

P = 128
C_MAX = 1536      # max token capacity per wave (SBUF-resident xt + act)

f32 = mybir.dt.float32
bf16 = mybir.dt.bfloat16
Silu = mybir.ActivationFunctionType.Silu
Identity = mybir.ActivationFunctionType.Identity
Alu = mybir.AluOpType

_cache = {}

# set by the most recent kernel() call when KERNEL_TRACE=1 (test harness use)
last_exec_time_ns = None
last_results = None


def _chunking(cmax):
    """Pick (chunk_width, n_chunks): CW <= 512 (PSUM bank), minimal padded
    capacity, CW large enough that weight loads stay hidden."""
    NCH = max(1, -(-cmax // 512))
    cw = max(256, ((-(-cmax // NCH)) + 1) // 2 * 2)
    return cw, NCH


def _build(CW, NCH, H, F, n_cores):
    """Build+schedule the per-core MLP program for token capacity CW*NCH."""
    C = CW * NCH
    F2 = 2 * F
    KT1 = H // P      # k-tiles of GEMM1 (16)
    FT = F2 // P      # f-tiles of W1    (44)
    FP = F // P       # f-tile pairs / k-tiles of GEMM2 (22)
    MT = H // P       # m-tiles of GEMM2 (16)

    nc = bacc.Bacc("TRN2", target_bir_lowering=False, debug=False,
                   num_devices=n_cores)

    xt_d = nc.dram_tensor("xt", (H, C), bf16, kind="ExternalInput").ap()
    # pre-tiled weights: w1t[ft, p, ko, fi] = w1[ko*P + p, ft*P + fi]
    w1_d = nc.dram_tensor("w1t", (FT, P, KT1, P), bf16, kind="ExternalInput").ap()
    # biases pre-tiled host-side to [P, n] so the DMA descriptors are
    # per-partition contiguous: b1t[p, ft] = b1[ft*P + p]
    b1_d = nc.dram_tensor("b1t", (P, FT), f32, kind="ExternalInput").ap()
    # w2t[mt, p, ko, mi] = w2[ko*P + p, mt*P + mi]  (bf16)
    w2_d = nc.dram_tensor("w2t", (MT, P, FP, P), bf16, kind="ExternalInput").ap()
    b2_d = nc.dram_tensor("b2t", (P, MT), f32, kind="ExternalInput").ap()
    yt_d = nc.dram_tensor("yt", (H, C), f32, kind="ExternalOutput").ap()

    xt_t = xt_d.rearrange("(ko p) c -> p ko c", p=P)    # [128, KT1, C]
    yt_t = yt_d.rearrange("(mt p) c -> p mt c", p=P)

    # Stagger chunk ch by LAG f-iterations behind chunk 0 so the startup
    # only waits on chunk 0's xt tiles + the first weight strips (the DMA
    # queues deliver xt chunk 1+ and later weights while PE is busy).
    # Smaller LAG at NCH>=3 keeps the w1 pair-pinning window (and so the
    # weight pool) small enough for SBUF. LAG=6 at NCH=2 so chunk 1's
    # first read (step LAG) comes after its group DMAs, which interleave
    # behind weight pairs 4..7 in trace order (Tile deps follow trace
    # order: a read emitted before the write would race).
    LAG = 6 if NCH <= 2 else 2
    order = []
    for step in range(FP + (NCH - 1) * LAG):
        for ch in range(NCH):
            f = step - ch * LAG
            if 0 <= f < FP:
                order.append((f, ch))

    with tile.TileContext(nc) as tc:
        with tc.tile_pool(name="persist", bufs=1) as persist, \
             tc.tile_pool(name="w1pool", bufs=2 * ((NCH - 1) * LAG + 3)) as w1pool, \
             tc.tile_pool(name="w2pool", bufs=2) as w2pool, \
             tc.tile_pool(name="evac", bufs=3) as evac, \
             tc.tile_pool(name="ps1", bufs=2, space="PSUM") as ps1, \
             tc.tile_pool(name="ps2", bufs=4, space="PSUM") as ps2:

            # xt tiles in groups of GK k-tiles per DMA (fewer serialized
            # trigger slots). Chunk 0 streams on the two otherwise-idle HWDGE
            # queues (sync + vector) at t=0; later chunks are only needed LAG
            # f-iterations in, so they queue up behind the first w1 strips on
            # the weight queue (see below).
            GK = 4
            NG = KT1 // GK
            xt_sb = {}
            xt_groups = {}
            for g in range(NG):
                t = persist.tile([P, GK, CW], bf16, tag=f"xtg_{g}_0")
                if g == 0:
                    # halved first piece so the first matmul chain starts
                    # early without starving on k=1
                    h2 = GK // 2
                    nc.sync.dma_start(t[:, :h2], xt_t[:, :h2, ts(0, CW)])
                    nc.sync.dma_start(t[:, h2:], xt_t[:, h2:GK, ts(0, CW)])
                else:
                    nc.sync.dma_start(t[:], xt_t[:, ts(g, GK), ts(0, CW)])
                xt_groups[g, 0] = t
                for j in range(GK):
                    xt_sb[g * GK + j, 0] = t[:, j]
            for ch in range(1, NCH):
                for g in range(NG):
                    t = persist.tile([P, GK, CW], bf16, tag=f"xtg_{g}_{ch}")
                    xt_groups[g, ch] = t
                    for j in range(GK):
                        xt_sb[g * GK + j, ch] = t[:, j]
            act_sb = persist.tile([P, FP, C], bf16)
            b1_sb = persist.tile([P, FT], f32)
            b2_sb = persist.tile([P, MT], f32)

            # ---- GEMM1 + SwiGLU, one f-tile pair (a, b halves) at a time
            with nc.named_scope("gemm1"):
                w1_sb = {}
                n_pairs = 0
                for fp, ch in order:
                    if fp not in w1_sb:
                        w1a = w1pool.tile([P, KT1, P], bf16, tag="w1s")
                        w1b = w1pool.tile([P, KT1, P], bf16, tag="w1s")
                        if n_pairs < 1:
                            # small leading piece of the first pair's strips
                            # so the first matmul chain starts as soon as
                            # k-tiles land; a/b interleaved to match the
                            # chain order below
                            for lo, hi in ((0, GK), (GK, KT1)):
                                nc.scalar.dma_start(w1a[:, lo:hi],
                                                    w1_d[fp][:, lo:hi])
                                nc.scalar.dma_start(w1b[:, lo:hi],
                                                    w1_d[FP + fp][:, lo:hi])
                        else:
                            nc.scalar.dma_start(w1a[:], w1_d[fp])
                            nc.scalar.dma_start(w1b[:], w1_d[FP + fp])
                        w1_sb[fp] = (w1a, w1b)
                        n_pairs += 1
                        if n_pairs == 1:
                            # biases ride behind the first pair (first needed
                            # by the fp=0 SwiGLU, ~15us in)
                            nc.sync.dma_start(b1_sb[:], b1_d)
                            nc.sync.dma_start(b2_sb[:], b2_d)
                        if NCH == 2:
                            # xt chunk 1 rides the weight queue, one group
                            # interleaved after each of pairs 4..4+NG-1 so it
                            # neither competes at t=0 nor delays the pairs PE
                            # consumes first (chunk 1 is first read ~LAG
                            # f-iterations in)
                            if 4 <= n_pairs < 4 + NG:
                                g = n_pairs - 4
                                nc.scalar.dma_start(
                                    xt_groups[g, 1][:],
                                    xt_t[:, ts(g, GK), ts(1, CW)])
                        elif n_pairs == 2:
                            # NCH>=3: chunk 1 is needed sooner (LAG=2), so
                            # all later chunks go right behind the first two
                            # pairs
                            for ch2 in range(1, NCH):
                                for g in range(NG):
                                    nc.scalar.dma_start(
                                        xt_groups[g, ch2][:],
                                        xt_t[:, ts(g, GK), ts(ch2, CW)])
                    w1a, w1b = w1_sb[fp]
                    pa = ps1.tile([P, CW], f32, tag="pa")
                    pb = ps1.tile([P, CW], f32, tag="pb")
                    if fp == 0 and ch == 0:
                        # startup: interleave the a/b chains per xt group so
                        # each arriving group feeds 2*GK matmuls and PE never
                        # outruns the DMA pipe
                        for g in range(NG):
                            for half, p_ in ((w1a, pa), (w1b, pb)):
                                for k in range(g * GK, (g + 1) * GK):
                                    nc.tensor.matmul(
                                        p_, half[:, k], xt_sb[k, ch][:],
                                        start=(k == 0), stop=(k == KT1 - 1))
                    else:
                        for k in range(KT1):
                            nc.tensor.matmul(pa, w1a[:, k],
                                             xt_sb[k, ch][:],
                                             start=(k == 0), stop=(k == KT1 - 1))
                        for k in range(KT1):
                            nc.tensor.matmul(pb, w1b[:, k],
                                             xt_sb[k, ch][:],
                                             start=(k == 0), stop=(k == KT1 - 1))
                    s = evac.tile([P, CW], f32, tag="silu")
                    nc.scalar.activation(s, pa, Silu,
                                         bias=b1_sb[:, fp:fp + 1])
                    # act = (pb + b1b) * silu(pa + b1a), cast bf16 on write
                    nc.vector.scalar_tensor_tensor(
                        act_sb[:, fp, ts(ch, CW)], pb,
                        b1_sb[:, FP + fp:FP + fp + 1], s,
                        Alu.add, Alu.mult)
                    if ch == NCH - 1:
                        del w1_sb[fp]

            # ---- GEMM2
            with nc.named_scope("gemm2"):
                for m in range(MT):
                    # same queue as w1 so the w2 prefetch queues up BEHIND the
                    # startup-critical w1 strips instead of competing at t=0
                    w2s = w2pool.tile([P, FP, P], bf16, tag="w2s")
                    nc.scalar.dma_start(w2s[:], w2_d[m])
                    for ch in range(NCH):
                        if m == MT - 1 and ch == NCH - 1:
                            # split the very last chunk into column halves so
                            # the first half's evac+DMA hides under the second
                            # half's matmuls, shortening the kernel tail
                            hw_ = CW // 2
                            for piece, (lo, w_) in enumerate(
                                    ((0, hw_), (hw_, CW - hw_))):
                                py = ps2.tile([P, w_], f32, tag="py")
                                for k in range(FP):
                                    nc.tensor.matmul(
                                        py, w2s[:, k],
                                        act_sb[:, k,
                                               ch * CW + lo:ch * CW + lo + w_],
                                        start=(k == 0), stop=(k == FP - 1))
                                y = evac.tile([P, w_], f32, tag="y")
                                nc.scalar.activation(y, py, Identity,
                                                     bias=b2_sb[:, m:m + 1])
                                eng = nc.sync if piece == 0 else nc.scalar
                                eng.dma_start(
                                    yt_t[:, m, ch * CW + lo:ch * CW + lo + w_],
                                    y[:])
                            continue
                        py = ps2.tile([P, CW], f32, tag="py")
                        for k in range(FP):
                            nc.tensor.matmul(py, w2s[:, k],
                                             act_sb[:, k, ts(ch, CW)],
                                             start=(k == 0), stop=(k == FP - 1))
                        y = evac.tile([P, CW], f32, tag="y")
                        nc.scalar.activation(y, py, Identity,
                                             bias=b2_sb[:, m:m + 1])
                        nc.sync.dma_start(yt_t[:, m, ts(ch, CW)], y[:])

    nc.compile()
    return nc


def kernel(hidden_states, expert_weights, w1, b1, w2, b2, top_experts):
    global last_exec_time_ns, last_results

    hidden_states = np.asarray(hidden_states)
    B, S, H = hidden_states.shape
    E, _, F2 = np.asarray(w1).shape
    F = F2 // 2
    topk = np.asarray(top_experts).shape[-1]
    N = B * S
    n_cores = 8
    assert E == n_cores, f"kernel assumes one expert per core, got E={E}"

    x = np.ascontiguousarray(hidden_states.reshape(N, H).astype(np.float32))
    te = np.asarray(top_experts).reshape(N, topk).astype(np.int64)
    ew = np.asarray(expert_weights).reshape(N, topk).astype(np.float32)

    # Dedup routing: token t needs expert e once, with combined weight
    # sum_j ew[t, j] * [te[t, j] == e].
    tok_lists = []
    wt_lists = []
    for e in range(E):
        sel = (te == e)                    # [N, topk]
        toks = np.nonzero(sel.any(axis=1))[0]
        wts = (ew[toks] * sel[toks]).sum(axis=1)
        tok_lists.append(toks)
        wt_lists.append(wts)
    counts = np.array([len(t) for t in tok_lists])

    cmax = int(counts.max())
    cw, nch = _chunking(min(max(cmax, 1), C_MAX))
    cap = cw * nch
    n_waves = max(1, -(-cmax // cap))

    key = (int(cw), int(nch), H, F, n_cores)
    if key not in _cache:
        _cache[key] = _build(*key)
    nc = _cache[key]

    # per-expert constant inputs (weights pre-tiled to contiguous SBUF strips)
    KT1, FT, FP, MT = H // P, F2 // P, F // P, H // P
    const_maps = []
    for e in range(E):
        w1e = np.asarray(w1[e], dtype=np.float32).astype(ml_dtypes.bfloat16)
        w1t = np.ascontiguousarray(
            w1e.reshape(KT1, P, FT, P).transpose(2, 1, 0, 3))
        w2e = np.asarray(w2[e], dtype=np.float32).astype(ml_dtypes.bfloat16)
        w2t = np.ascontiguousarray(
            w2e.reshape(FP, P, MT, P).transpose(2, 1, 0, 3))
        const_maps.append({
            "w1t": w1t,
            "b1t": np.ascontiguousarray(
                np.asarray(b1[e], dtype=np.float32).reshape(FT, P).T),
            "w2t": w2t,
            "b2t": np.ascontiguousarray(
                np.asarray(b2[e], dtype=np.float32).reshape(MT, P).T),
        })

    trace = os.environ.get("KERNEL_TRACE", "") == "1"
    out = np.zeros((N, H), dtype=np.float32)
    last_results = []
    for w in range(n_waves):
        in_maps = []
        for e in range(E):
            lo = w * cap
            toks = tok_lists[e][lo: lo + cap]
            xt = np.zeros((H, cap), dtype=ml_dtypes.bfloat16)
            if len(toks):
                xt[:, :len(toks)] = x[toks].T.astype(ml_dtypes.bfloat16)
            in_maps.append({"xt": xt, **const_maps[e]})
        tmpdir = None
        if trace:
            import shutil
            tmpdir = f"/tmp/moe_trace_w{w}"
            shutil.rmtree(tmpdir, ignore_errors=True)
            os.makedirs(tmpdir, exist_ok=True)
        res = bass_utils.run_bass_kernel_spmd(
            nc, in_maps, core_ids=list(range(n_cores)), trace=trace,
            tmpdir=tmpdir)
        last_results.append(res)
        if trace:
            last_exec_time_ns = res.exec_time_ns
        for e in range(E):
            lo = w * cap
            toks = tok_lists[e][lo: lo + cap]
            if len(toks):
                yt = res.results[e]["yt"]
                wts = wt_lists[e][lo: lo + cap]
                out[toks] += wts[:, None] * yt[:, :len(toks)].T

    return out.reshape(B, S, H).astype(np.float32)
